# revision 14
# baseline (speedup 1.0000x reference)
"""KAN layer (Chebyshev deg-8) Trainium2 kernel, 8-core data-parallel.

Math: out[b] = sum_n hw[n] * (X @ C.T)[b,n] = X[b,:] @ (C.T @ hw)
            = sum_d sum_k W[d,k] * T_k(tanh(x[b,d])),  W[d,k]=(C.T@hw)[d*9+k]

Device evaluates a centered-monomial basis in fp16 (all ops global — no
per-dim parameters — so both 128-dim chunks share every elementwise op on
fused [128, 2*BLK] tiles):
  e1=u, e2=q=u^2, e3=u*q, e4=P2=(q-1/2)^2, e5=u*P2, e6=q*P2, e7=u*q*P2,
  e8=P4=(P2-1/8)^2         (note T8(u) = 128*P4 - 1 exactly)
The centering biases ride free on ACT's Square (out = f(in*scale+bias));
products run on DVE at 2x fp16; the split is LP-balanced (ACT ~ DVE).
The d-contraction runs on the PE as per-stream matvecs with 4 batch
sub-blocks concurrent via column tiling (PSUM rows 0/32/64/96 of one bank).
Host: transposes x to [D, B] fp16, folds hweights into coeffs, solves the
stream->Chebyshev transform with fp16-rounding compensation, and adds the
T0 constant plus output-row gather on the way out.
"""
import sys
import numpy as np

sys.path.insert(0, "/opt/trn_rl_repo")

import orjson
from contextlib import ExitStack

import concourse.bass as bass
from concourse import mybir
from concourse.tile import TileContext
from concourse.bass_utils import run_bass_kernel_spmd

F32 = mybir.dt.float32
F16 = mybir.dt.float16
AF = mybir.ActivationFunctionType
OP = mybir.AluOpType

B, D, DEG1 = 32768, 256, 9
NCORES = 8
BC = B // NCORES          # 4096 batch per core
BLK = 2048                # batch cols per super-block
NSB = BC // BLK           # super-blocks per core
FUS = 2 * BLK             # fused free dim: [0,BLK)=dims 0-127, [BLK,2BLK)=dims 128-255
NGRP = 4                  # PE column groups (batch sub-blocks in flight)
SUB = BLK // NGRP         # 512 cols per sub-block == one PSUM bank row

STREAMS = ["u", "q", "uq", "P2", "uP2", "qP2", "uqP2", "P4"]
MM_ORDER = [0, 1, 2, 3, 4, 5, 7, 6]   # by availability: u,q,uq,P2,uP2,qP2,P4,uqP2

# ---- walrus workaround: split >1 sem-waits onto Drain carriers -------------
_MAXW = 1

def _split_waits(bir_json: bytes) -> bytes:
    d = orjson.loads(bir_json)
    for fn in d.get("functions", []):
        for bb in fn.get("blocks", []):
            out = []
            for ins in bb.get("instructions", []):
                si = ins.get("sync_info") or {}
                waits = si.get("on_wait") or []
                if len(waits) > _MAXW:
                    extra, keep = waits[:-_MAXW], waits[-_MAXW:]
                    for i in range(0, len(extra), _MAXW):
                        out.append({
                            "debug": ins.get("debug", 0),
                            "engine": ins["engine"], "ins": [], "outs": [],
                            "name": f"{ins['name']}_ws{i}", "opcode": "Drain",
                            "sync_info": {"on_update": [],
                                          "on_wait": extra[i:i + _MAXW]},
                        })
                    si["on_wait"] = keep
                out.append(ins)
            bb["instructions"] = out
    return orjson.dumps(d)

def _install_patch():
    import concourse.bass_utils as bu
    if getattr(bu, "_ws_patched", False):
        return
    orig = bu.compile_bir_kernel
    def patched(bir_json, tmpdir, neff_name="file.neff"):
        return orig(_split_waits(bir_json), tmpdir, neff_name)
    bu.compile_bir_kernel = patched
    bu._ws_patched = True
    try:
        import concourse.bass2jax as b2j
        if getattr(b2j, "compile_bir_kernel", None) is orig:
            b2j.compile_bir_kernel = patched
    except Exception:
        pass

# ---- basis transform (host) ------------------------------------------------
def _stream_polys():
    """Power-basis coefficients (in u) of each stream, index by degree 1..8."""
    P = np.polynomial.polynomial
    u = [0.0, 1.0]
    q = P.polymul(u, u)
    uq = P.polymul(u, q)
    p2 = P.polymul(P.polyadd(q, [-0.5]), P.polyadd(q, [-0.5]))
    up2 = P.polymul(u, p2)
    qp2 = P.polymul(q, p2)
    uqp2 = P.polymul(uq, p2)
    p4 = P.polymul(P.polyadd(p2, [-0.125]), P.polyadd(p2, [-0.125]))
    return {1: u, 2: q, 3: uq, 4: p2, 5: up2, 6: qp2, 7: uqp2, 8: p4}

def _basis_matrix():
    """A[:, t] = Chebyshev T_0..T_8 coefficients of the degree-t stream."""
    from numpy.polynomial import chebyshev as C
    A = np.zeros((9, 9))
    A[0, 0] = 1.0
    for t, poly in _stream_polys().items():
        c = C.poly2cheb(poly)
        A[: len(c), t] = c
    return A

# ---- device kernel ---------------------------------------------------------
def _build(strided_out=True):
    nc = bass.Bass()
    xt = nc.declare_dram_parameter("xt", [D, BC], F16, isOutput=False)
    wv = nc.declare_dram_parameter("wv", [128, 16], F16, isOutput=False)
    y = nc.declare_dram_parameter("y", [NGRP, NSB * SUB], F32, isOutput=True)

    with TileContext(nc) as tc, ExitStack() as ctx:
        cpool = ctx.enter_context(tc.tile_pool(name="const", bufs=1))
        fp = ctx.enter_context(tc.tile_pool(name="feat", bufs=2))
        pp = ctx.enter_context(tc.tile_pool(name="ps", bufs=2, space="PSUM"))

        # first compute tile's input issued ahead of everything else
        xfs = []
        for sb in range(NSB):
            bs = sb * BLK
            xf = fp.tile([128, FUS], F16, tag="x", name=f"xf{sb}")
            nc.sync.dma_start(out=xf[:, 0:BLK], in_=xt[0:128, bs:bs + BLK])
            nc.sync.dma_start(out=xf[:, BLK:FUS], in_=xt[128:256, bs:bs + BLK])
            xfs.append(xf)

        wb = cpool.tile([128, 16], F16)
        nc.sync.dma_start(out=wb[:], in_=wv[:])
        bH = cpool.tile([128, 1], F32)
        nc.vector.memset(bH[:], -0.5)
        bE = cpool.tile([128, 1], F32)
        nc.vector.memset(bE[:], -0.125)

        res = cpool.tile([128, NSB * SUB], F32)
        psts = []
        for sb in range(NSB):
            xf = xfs[sb]
            u = fp.tile([128, FUS], F16, tag="u")
            nc.scalar.activation(u[:], xf[:], AF.Tanh)
            q = fp.tile([128, FUS], F16, tag="q")
            nc.vector.tensor_mul(q[:], u[:], u[:])
            P2 = fp.tile([128, FUS], F16, tag="P2")
            nc.scalar.activation(P2[:], q[:], AF.Square, bias=bH[:])
            uq = fp.tile([128, FUS], F16, tag="uq")
            nc.vector.tensor_mul(uq[:], u[:], q[:])
            P4 = fp.tile([128, FUS], F16, tag="P4")
            nc.scalar.activation(P4[:], P2[:], AF.Square, bias=bE[:])
            uP2 = fp.tile([128, FUS], F16, tag="uP2")
            nc.vector.tensor_mul(uP2[:], u[:], P2[:])
            qP2 = fp.tile([128, FUS], F16, tag="qP2")
            nc.vector.tensor_mul(qP2[:], q[:], P2[:])
            uqP2 = fp.tile([128, FUS], F16, tag="uqP2")
            nc.vector.tensor_mul(uqP2[:], uq[:], P2[:])
            tiles = dict(u=u, q=q, uq=uq, P2=P2, uP2=uP2,
                         qP2=qP2, uqP2=uqP2, P4=P4)

            ps = pp.tile([128, SUB], F32)
            psts.append(ps)
            nround = 2 * len(MM_ORDER)
            r = 0
            for sidx in MM_ORDER:
                for c in range(2):
                    stream = tiles[STREAMS[sidx]]
                    for g in range(NGRP):
                        nc.tensor.matmul(
                            ps[32 * g:32 * g + 1, :],
                            wb[:, c * 8 + sidx:c * 8 + sidx + 1],
                            stream[:, c * BLK + g * SUB:c * BLK + (g + 1) * SUB],
                            start=(r == 0), stop=(r == nround - 1),
                            skip_group_check=True,
                            tile_position=(0, 32 * g))
                    r += 1

        for sb in range(NSB):
            dst = res[:, sb * SUB:(sb + 1) * SUB]
            if sb % 2 == 0:
                nc.scalar.activation(dst, psts[sb][:], AF.Identity)
            else:
                nc.vector.tensor_copy(dst, psts[sb][:])
        if strided_out:
            nc.sync.dma_start(out=y[:], in_=res[0:128:32, :])
        else:
            for g in range(NGRP):
                nc.sync.dma_start(out=y[g:g + 1, :], in_=res[32 * g:32 * g + 1, :])
    return nc

# ---- public entry ----------------------------------------------------------
def kernel(x, coeffs, hweights, _trace=False):
    _install_patch()
    x = np.asarray(x, dtype=np.float32)
    w = (coeffs.astype(np.float64).T @ hweights.astype(np.float64))  # [2304]
    W = w.reshape(D, DEG1)                                           # [d, k]
    # quantization-compensated solve: peel leading Chebyshev components in
    # decreasing degree; each stream's fp16 weight rounding is re-absorbed by
    # the lower-degree streams, leftover T0 becomes the host-side constant.
    A = _basis_matrix()
    Wc = W.astype(np.float64).copy()
    lam = np.zeros((D, DEG1))
    for t in range(DEG1 - 1, 0, -1):
        lt = Wc[:, t] / A[t, t]
        ltq = lt.astype(np.float16).astype(np.float64)
        Wc -= ltq[:, None] * A[:, t][None, :]
        lam[:, t] = ltq
    c0 = float(Wc[:, 0].sum())
    wv = np.zeros((128, 16), dtype=np.float16)
    for c in range(2):
        for sidx in range(8):
            wv[:, c * 8 + sidx] = lam[c * 128:(c + 1) * 128, sidx + 1]

    nc = _build()
    xT = np.ascontiguousarray(x.T.astype(np.float16))                # [D, B]
    in_maps = [{"xt": np.ascontiguousarray(xT[:, i * BC:(i + 1) * BC]),
                "wv": wv} for i in range(NCORES)]
    res = run_bass_kernel_spmd(nc, in_maps, core_ids=list(range(NCORES)),
                               trace=_trace)
    # y[g, sb*SUB + i] holds batch col sb*BLK + g*SUB + i of this core;
    # the T0 constant c0 is added here (it is global, so host-side is free)
    parts = []
    for i in range(NCORES):
        yc = res.results[i]["y"].astype(np.float64) + c0   # [NGRP, NSB*SUB]
        parts.append(yc.reshape(NGRP, NSB, SUB).transpose(1, 0, 2).reshape(BC))
    out = np.concatenate(parts)
    if _trace:
        kernel._last = res
    return out.astype(np.float32)


# revision 17
# speedup vs baseline: 1.2254x; 1.2254x over previous
"""KAN layer (Chebyshev deg-8) Trainium2 kernel, 8-core data-parallel.

Math: out[b] = sum_n hw[n] * (X @ C.T)[b,n] = X[b,:] @ (C.T @ hw)
            = sum_d sum_k W[d,k] * T_k(tanh(x[b,d])),  W[d,k]=(C.T@hw)[d*9+k]

Device evaluates a centered-monomial basis in fp16 (all ops global — no
per-dim parameters — so both 128-dim chunks share every elementwise op on
fused [128, 2*BLK] tiles):
  e1=u, e2=q=u^2, e3=u*q, e4=P2=(q-1/2)^2, e5=u*P2, e6=q*P2, e7=u*q*P2,
  e8=P4=(P2-1/8)^2         (note T8(u) = 128*P4 - 1 exactly)
The centering biases ride free on ACT's Square (out = f(in*scale+bias));
products run on DVE at 2x fp16; the split is LP-balanced (ACT ~ DVE).
The d-contraction runs on the PE as per-stream matvecs with 4 batch
sub-blocks concurrent via column tiling (PSUM rows 0/32/64/96 of one bank).
Host: transposes x to [D, B] fp16, folds hweights into coeffs, solves the
stream->Chebyshev transform with fp16-rounding compensation, and adds the
T0 constant plus output-row gather on the way out.
"""
import sys
import numpy as np

sys.path.insert(0, "/opt/trn_rl_repo")

import orjson
from contextlib import ExitStack

import concourse.bass as bass
from concourse import mybir
from concourse.tile import TileContext
from concourse.bass_utils import run_bass_kernel_spmd

F32 = mybir.dt.float32
F16 = mybir.dt.float16
AF = mybir.ActivationFunctionType
OP = mybir.AluOpType

B, D, DEG1 = 32768, 256, 9
NCORES = 8
BC = B // NCORES          # 4096 batch per core
BLK = 2048                # batch cols per super-block
NSB = BC // BLK           # super-blocks per core
FUS = 2 * BLK             # fused free dim: [0,BLK)=dims 0-127, [BLK,2BLK)=dims 128-255
NGRP = 4                  # PE column groups (batch sub-blocks in flight)
SUB = BLK // NGRP         # 512 cols per sub-block == one PSUM bank row

STREAMS = ["u", "q", "uq", "P2", "uP2", "qP2", "uqP2", "P4"]
MM_ORDER = [0, 1, 2, 3, 4, 5, 7, 6]   # by availability: u,q,uq,P2,uP2,qP2,P4,uqP2

# ---- walrus workaround: split >1 sem-waits onto Drain carriers -------------
_MAXW = 1

def _split_waits(bir_json: bytes) -> bytes:
    d = orjson.loads(bir_json)
    for fn in d.get("functions", []):
        for bb in fn.get("blocks", []):
            out = []
            for ins in bb.get("instructions", []):
                si = ins.get("sync_info") or {}
                waits = si.get("on_wait") or []
                if len(waits) > _MAXW:
                    extra, keep = waits[:-_MAXW], waits[-_MAXW:]
                    for i in range(0, len(extra), _MAXW):
                        out.append({
                            "debug": ins.get("debug", 0),
                            "engine": ins["engine"], "ins": [], "outs": [],
                            "name": f"{ins['name']}_ws{i}", "opcode": "Drain",
                            "sync_info": {"on_update": [],
                                          "on_wait": extra[i:i + _MAXW]},
                        })
                    si["on_wait"] = keep
                out.append(ins)
            bb["instructions"] = out
    return orjson.dumps(d)

def _install_patch():
    import concourse.bass_utils as bu
    if getattr(bu, "_ws_patched", False):
        return
    orig = bu.compile_bir_kernel
    def patched(bir_json, tmpdir, neff_name="file.neff"):
        return orig(_split_waits(bir_json), tmpdir, neff_name)
    bu.compile_bir_kernel = patched
    bu._ws_patched = True
    try:
        import concourse.bass2jax as b2j
        if getattr(b2j, "compile_bir_kernel", None) is orig:
            b2j.compile_bir_kernel = patched
    except Exception:
        pass

# ---- basis transform (host) ------------------------------------------------
def _stream_polys():
    """Power-basis coefficients (in u) of each stream, index by degree 1..8."""
    P = np.polynomial.polynomial
    u = [0.0, 1.0]
    q = P.polymul(u, u)
    uq = P.polymul(u, q)
    p2 = P.polymul(P.polyadd(q, [-0.5]), P.polyadd(q, [-0.5]))
    up2 = P.polymul(u, p2)
    qp2 = P.polymul(q, p2)
    uqp2 = P.polymul(uq, p2)
    p4 = P.polymul(P.polyadd(p2, [-0.125]), P.polyadd(p2, [-0.125]))
    return {1: u, 2: q, 3: uq, 4: p2, 5: up2, 6: qp2, 7: uqp2, 8: p4}

def _basis_matrix():
    """A[:, t] = Chebyshev T_0..T_8 coefficients of the degree-t stream."""
    from numpy.polynomial import chebyshev as C
    A = np.zeros((9, 9))
    A[0, 0] = 1.0
    for t, poly in _stream_polys().items():
        c = C.poly2cheb(poly)
        A[: len(c), t] = c
    return A

# ---- device kernel ---------------------------------------------------------
def _build(strided_out=True):
    nc = bass.Bass()
    xt = nc.declare_dram_parameter("xt", [D, BC], F16, isOutput=False)
    wv = nc.declare_dram_parameter("wv", [128, 16], F16, isOutput=False)
    y = nc.declare_dram_parameter("y", [NGRP, NSB * SUB], F32, isOutput=True)

    with TileContext(nc) as tc, ExitStack() as ctx:
        cpool = ctx.enter_context(tc.tile_pool(name="const", bufs=1))
        fp = ctx.enter_context(tc.tile_pool(name="feat", bufs=4))
        pp = ctx.enter_context(tc.tile_pool(name="ps", bufs=2, space="PSUM"))

        # compute tiles' inputs issued ahead of everything else, in use order
        xfs = {}
        for sb in range(NSB):
            bs = sb * BLK
            for c in range(2):
                xf = fp.tile([128, BLK], F16, tag="x", name=f"xf{c}_{sb}")
                nc.sync.dma_start(out=xf[:], in_=xt[128 * c:128 * (c + 1), bs:bs + BLK])
                xfs[(c, sb)] = xf

        wb = cpool.tile([128, 16], F16)
        nc.sync.dma_start(out=wb[:], in_=wv[:])
        bH = cpool.tile([128, 1], F32)
        nc.vector.memset(bH[:], -0.5)
        bE = cpool.tile([128, 1], F32)
        nc.vector.memset(bE[:], -0.125)

        res = cpool.tile([128, NSB * SUB], F32)
        psts = []
        tiles = {}
        for sb in range(NSB):
            for c in range(2):
                xf = xfs[(c, sb)]
                u = fp.tile([128, BLK], F16, tag="u")
                nc.scalar.activation(u[:], xf[:], AF.Tanh)
                q = fp.tile([128, BLK], F16, tag="q")
                nc.vector.tensor_mul(q[:], u[:], u[:])
                P2 = fp.tile([128, BLK], F16, tag="P2")
                nc.scalar.activation(P2[:], q[:], AF.Square, bias=bH[:])
                uq = fp.tile([128, BLK], F16, tag="uq")
                nc.vector.tensor_mul(uq[:], u[:], q[:])
                P4 = fp.tile([128, BLK], F16, tag="P4")
                nc.scalar.activation(P4[:], P2[:], AF.Square, bias=bE[:])
                uP2 = fp.tile([128, BLK], F16, tag="uP2")
                nc.vector.tensor_mul(uP2[:], u[:], P2[:])
                qP2 = fp.tile([128, BLK], F16, tag="qP2")
                nc.vector.tensor_mul(qP2[:], q[:], P2[:])
                uqP2 = fp.tile([128, BLK], F16, tag="uqP2")
                nc.vector.tensor_mul(uqP2[:], uq[:], P2[:])
                tiles[(c, sb)] = dict(u=u, q=q, uq=uq, P2=P2, uP2=uP2,
                                      qP2=qP2, uqP2=uqP2, P4=P4)

            ps = pp.tile([128, SUB], F32)
            psts.append(ps)
            nround = 2 * len(MM_ORDER)
            r = 0
            for sidx in MM_ORDER:
                for c in range(2):
                    stream = tiles[(c, sb)][STREAMS[sidx]]
                    for g in range(NGRP):
                        nc.tensor.matmul(
                            ps[32 * g:32 * g + 1, :],
                            wb[:, c * 8 + sidx:c * 8 + sidx + 1],
                            stream[:, g * SUB:(g + 1) * SUB],
                            start=(r == 0), stop=(r == nround - 1),
                            skip_group_check=True,
                            tile_position=(0, 32 * g))
                    r += 1

        for sb in range(NSB):
            dst = res[:, sb * SUB:(sb + 1) * SUB]
            if sb % 2 == 0:
                nc.scalar.activation(dst, psts[sb][:], AF.Identity)
            else:
                nc.vector.tensor_copy(dst, psts[sb][:])
        if strided_out:
            nc.sync.dma_start(out=y[:], in_=res[0:128:32, :])
        else:
            for g in range(NGRP):
                nc.sync.dma_start(out=y[g:g + 1, :], in_=res[32 * g:32 * g + 1, :])
    return nc

# ---- public entry ----------------------------------------------------------
def kernel(x, coeffs, hweights, _trace=False):
    _install_patch()
    x = np.asarray(x, dtype=np.float32)
    w = (coeffs.astype(np.float64).T @ hweights.astype(np.float64))  # [2304]
    W = w.reshape(D, DEG1)                                           # [d, k]
    # quantization-compensated solve: peel leading Chebyshev components in
    # decreasing degree; each stream's fp16 weight rounding is re-absorbed by
    # the lower-degree streams, leftover T0 becomes the host-side constant.
    A = _basis_matrix()
    Wc = W.astype(np.float64).copy()
    lam = np.zeros((D, DEG1))
    for t in range(DEG1 - 1, 0, -1):
        lt = Wc[:, t] / A[t, t]
        ltq = lt.astype(np.float16).astype(np.float64)
        Wc -= ltq[:, None] * A[:, t][None, :]
        lam[:, t] = ltq
    c0 = float(Wc[:, 0].sum())
    wv = np.zeros((128, 16), dtype=np.float16)
    for c in range(2):
        for sidx in range(8):
            wv[:, c * 8 + sidx] = lam[c * 128:(c + 1) * 128, sidx + 1]

    nc = _build()
    xT = np.ascontiguousarray(x.T.astype(np.float16))                # [D, B]
    in_maps = [{"xt": np.ascontiguousarray(xT[:, i * BC:(i + 1) * BC]),
                "wv": wv} for i in range(NCORES)]
    res = run_bass_kernel_spmd(nc, in_maps, core_ids=list(range(NCORES)),
                               trace=_trace)
    # y[g, sb*SUB + i] holds batch col sb*BLK + g*SUB + i of this core;
    # the T0 constant c0 is added here (it is global, so host-side is free)
    parts = []
    for i in range(NCORES):
        yc = res.results[i]["y"].astype(np.float64) + c0   # [NGRP, NSB*SUB]
        parts.append(yc.reshape(NGRP, NSB, SUB).transpose(1, 0, 2).reshape(BC))
    out = np.concatenate(parts)
    if _trace:
        kernel._last = res
    return out.astype(np.float32)
